# revision 19
# baseline (speedup 1.0000x reference)
"""Trainium2 Bass kernel for nn_LDRFat (3-layer MLP forward).

reference: logits = relu((x @ W) @ fc_w.T + fc_b) @ logits_w.T + logits_b

Algebraic optimization: (x @ W) @ fc_w.T == x @ (W @ fc_w.T).
Precomputing Wfc = W @ fc_w.T ([3072,512]) collapses the dominant
309 GFLOP x@W matmul into a 51.5 GFLOP x@Wfc.

Layout strategy: ALL transposes/packing happen on the host (numpy)
inside kernel() — the device graph is a pure matmul stream (keeps the
PE HAM clock gate warm; transposes would idle it). Inputs are packed
into SBUF-layout panels [128, ...] so every DMA moves >=2.3KB
contiguous lines.

Phase A: each core computes its 384 rows of Wfc = W @ fc_w.T
(72 accumulating bf16 matmuls), then a bf16 AllGather replicates the
full Wfc. Phase B: h^T tiles [f, m] via matmuls contracted over n,
ReLU+bias on the scalar engine, then a 4-matmul logits head with
lgwT stationary producing out^T [10, m]; logits bias is applied as a
per-partition bias in the scalar-engine PSUM->SBUF copy. The final
output transpose ([10, BS] -> [BS, 10]) happens on the host.

Scheduling: the sync-engine DMA rings carry only phase-A inputs and x
chunks 0-1 up front, so the collective's input lands on idle rings
right after phase A. x chunks 2-3 and the gather-out unpacking ride
the activation engine's rings after the collective; the gather-in is
split per source core so phase B starts as soon as block 0 lands.
Dummy matmuls at t=0 pre-warm the PE clock (HAM gate).

All matmul operands are bf16 (fp32 PSUM accumulation): same PE rate as
fp32r but half the DMA/SBUF/collective bytes. Measured rel err ~5e-3
vs the 2e-2 gate.
"""

import os
import numpy as np
import ml_dtypes

import concourse.bass as bass
import concourse.mybir as mybir
import concourse.tile as tile
from concourse import bacc
from concourse.bass import MemorySpace, ts, ds
from concourse.bass_utils import run_bass_kernel_spmd

B = 16384
N = 3072
FC = 512
CLS = 10
NCORES = 8
BS = B // NCORES     # 2048 batch rows per core
P = 128

KT = N // P          # 24 contraction tiles
FT = FC // P         # 4 f-tiles
MCHUNK = 512
NMC = BS // MCHUNK   # 4 m-chunks per core
KSH = KT // NCORES   # 3 k-tiles per core in sharded precompute
WROWS = KSH * P      # 384 W-rows per core
NWARM = 24           # PE pre-warm matmuls

F32 = mybir.dt.float32
BF16 = mybir.dt.bfloat16
BF = ml_dtypes.bfloat16

_CACHE = {}
LAST_RESULT = None


def build_kernel():
    nc = bacc.Bacc(
        "TRN2",
        target_bir_lowering=False,
        debug=False,
        enable_asserts=False,
        num_devices=NCORES,
    )
    # All big inputs are host-packed to SBUF layout: [128 partitions, flat]
    xP_d = nc.dram_tensor("xP", [P, NMC * KT * MCHUNK], BF16, kind="ExternalInput").ap()
    wP_d = nc.dram_tensor("wP", [P, KT * WROWS], BF16, kind="ExternalInput").ap()
    fwP_d = nc.dram_tensor("fwP", [P, KT * FC], BF16, kind="ExternalInput").ap()
    fcb_d = nc.dram_tensor("fc_b", [FC], F32, kind="ExternalInput").ap()
    lwT_d = nc.dram_tensor("lgwT", [FC, CLS], BF16, kind="ExternalInput").ap()
    lgb_d = nc.dram_tensor("lgb", [CLS, 1], F32, kind="ExternalInput").ap()
    outT_d = nc.dram_tensor("outT", [CLS, BS], F32, kind="ExternalOutput").ap()

    with tile.TileContext(nc) as tc:
        with (
            tc.tile_pool(name="consts", bufs=1) as consts,
            tc.tile_pool(name="big", bufs=1) as big,
            tc.tile_pool(name="stage", bufs=1) as stage_pool,
            tc.tile_pool(name="yT_p", bufs=2) as yT_pool,
            tc.tile_pool(name="ps_a", bufs=2, space=MemorySpace.PSUM) as ps_a,
            tc.tile_pool(name="ps_b", bufs=4, space=MemorySpace.PSUM) as ps_b,
            tc.tile_pool(name="ps_lg", bufs=2, space=MemorySpace.PSUM) as ps_lg,
            tc.tile_pool(name="cc_dram", bufs=1, space=MemorySpace.DRAM) as ccd,
        ):
            fcb_sb = consts.tile([P, FT], F32)
            nc.sync.dma_start(fcb_sb, fcb_d.rearrange("(t p) -> p t", p=P))
            lgw_sb = consts.tile([P, FT, CLS], BF16)
            nc.sync.dma_start(lgw_sb, lwT_d.rearrange("(t p) c -> p t c", p=P))
            lgb_sb = consts.tile([CLS, 1], F32)
            nc.sync.dma_start(lgb_sb, lgb_d)

            # phase A inputs get the HBM to themselves (they gate the
            # collective, which gates phase B). All x loads are gated
            # behind gin below, so they stream during the collective's
            # dead window instead of competing with these.
            fwT_sb = big.tile([P, KT, FC], BF16)
            wT_sb = big.tile([P, KT, WROWS], BF16)
            for g in range(8):
                nc.sync.dma_start(
                    fwT_sb[:, 3 * g : 3 * g + 3],
                    fwP_d[:, ds(g * 3 * FC, 3 * FC)],
                )
                nc.sync.dma_start(
                    wT_sb[:, 3 * g : 3 * g + 3],
                    wP_d[:, ds(g * 3 * WROWS, 3 * WROWS)],
                )
            x_sb = [big.tile([P, KT, MCHUNK], BF16, name=f"x_{mc}") for mc in range(NMC)]
            XC = KT * MCHUNK  # flat elems per chunk

            def x_load(mc):
                for h in range(2):
                    nc.sync.dma_start(
                        x_sb[mc][:, 12 * h : 12 * h + 12],
                        xP_d[:, ds(mc * XC + h * XC // 2, XC // 2)],
                    )

            # First half of x chunk 0 loads with the inputs: the
            # collective trigger requires total DMA quiescence (EQ-based
            # semaphore waits), so anything pre-collective must fully
            # drain first — keep it to the minimum phase B needs
            # immediately (k-tiles 0-11 of chunk 0).
            nc.sync.dma_start(x_sb[0][:, 0:12], xP_d[:, ds(0, XC // 2)])

            # PE pre-warm: dummy matmuls (no data deps) fill the HAM
            # activity window so phase A runs at full clock.
            warm_f = consts.tile([P, MCHUNK], F32)
            nc.gpsimd.memset(warm_f, 0.01)
            warm_b = consts.tile([P, MCHUNK], BF16)
            nc.vector.tensor_copy(warm_b, warm_f)
            for i in range(NWARM):
                pw = ps_b.tile([P, MCHUNK], F32, tag="acc", name=f"warm_{i}")
                nc.tensor.matmul(pw, warm_b[:, :P], warm_b, start=True, stop=True)

            # ---------------- Phase A: this core's Wfc rows ----------------
            wfc_sb = big.tile([P, KT, FC], BF16)
            wfc_stage = stage_pool.tile([P, KSH, FC], BF16)
            for lkt in range(KSH):
                acc = ps_a.tile([P, FC], F32, tag="acc")
                for nt in range(KT):
                    nc.tensor.matmul(
                        acc,
                        wT_sb[:, nt, ts(lkt, P)],
                        fwT_sb[:, nt],
                        start=(nt == 0),
                        stop=(nt == KT - 1),
                    )
                nc.vector.tensor_copy(wfc_stage[:, lkt], acc)

            gin = ccd.tile([P, KSH * FC], BF16)
            nc.scalar.dma_start(gin, wfc_stage.rearrange("p a b -> p (a b)"))
            gout = ccd.tile([NCORES * P, KSH * FC], BF16)
            nc.gpsimd.collective_compute(
                "AllGather",
                mybir.AluOpType.bypass,
                replica_groups=[list(range(NCORES))],
                ins=[gin.opt()],
                outs=[gout.opt()],
            )
            # unpack per source core: block c holds global k-tiles
            # 3c..3c+2; phase B consumes kt in order, so it can start
            # as soon as block 0 lands.
            for c in range(NCORES):
                nc.scalar.dma_start(
                    wfc_sb[:, ds(KSH * c, KSH)].rearrange("p l f -> p (l f)"),
                    gout[ds(c * P, P), :],
                )
            # rest of x chunk 0 + chunk 1 stream once the collective is done
            gate1 = ccd.tile([1, 2], BF16, name="gate1")
            nc.sync.dma_start(gate1, gout[0:1, 0:2])
            nc.sync.dma_start(x_sb[0][:, 12:24], xP_d[:, ds(XC // 2, XC // 2)])
            x_load(1)

            # re-warm the PE clock just before phase B: these dummies
            # depend on gather-in block 0, so they run right as phase B
            # becomes unblocked and absorb the cold-clock window.
            for i in range(8):
                pw = ps_b.tile([P, MCHUNK], F32, tag="acc", name=f"warm2_{i}")
                nc.tensor.matmul(pw, wfc_sb[:, 0, 0:P], warm_b, start=True, stop=True)

            # ---------------- Phase B ----------------
            for mc in range(NMC):
                h2 = [
                    ps_b.tile([P, MCHUNK], F32, tag="acc", name=f"h2_{mc}_{ft}")
                    for ft in range(FT)
                ]
                for kt in range(KT):
                    for ft in range(FT):
                        nc.tensor.matmul(
                            h2[ft],
                            wfc_sb[:, kt, ts(ft, P)],
                            x_sb[mc][:, kt],
                            start=(kt == 0),
                            stop=(kt == KT - 1),
                        )

                # relu(h2 + fc_b) on the scalar engine, cast to bf16
                yT = yT_pool.tile([P, FT, MCHUNK], BF16, tag="yT")
                for ft in range(FT):
                    nc.scalar.activation(
                        yT[:, ft],
                        h2[ft],
                        mybir.ActivationFunctionType.Relu,
                        bias=fcb_sb[:, ds(ft, 1)],
                    )
                # release x chunk mc+2 while chunk mc+1 computes
                if mc + 2 < NMC:
                    gate = ccd.tile([1, 2], BF16, name=f"gate{mc + 2}")
                    nc.sync.dma_start(gate, yT[0:1, 0, 0:2])
                    x_load(mc + 2)

                # logits head: lgwT stationary, yT moving -> outT [CLS, m]
                plg = ps_lg.tile([CLS, MCHUNK], F32, tag="lg")
                for ft in range(FT):
                    nc.tensor.matmul(
                        plg,
                        lgw_sb[:, ft],
                        yT[:, ft],
                        start=(ft == 0),
                        stop=(ft == FT - 1),
                    )
                # + logits_b as per-partition bias during PSUM->SBUF copy
                osb = yT_pool.tile([CLS, MCHUNK], F32, tag="osb")
                nc.scalar.activation(
                    osb,
                    plg,
                    mybir.ActivationFunctionType.Identity,
                    bias=lgb_sb[:, ds(0, 1)],
                )
                nc.scalar.dma_start(outT_d[:, ts(mc, MCHUNK)], osb)

    nc.compile()
    return nc


def kernel(**inputs) -> np.ndarray:
    global LAST_RESULT
    if "nc" not in _CACHE:
        _CACHE["nc"] = build_kernel()
    nc = _CACHE["nc"]

    x = np.asarray(inputs["x"], dtype=np.float32)
    W = np.asarray(inputs["W"], dtype=np.float32)
    fc_w = np.asarray(inputs["fc_w"], dtype=np.float32)
    fc_b = np.asarray(inputs["fc_b"], dtype=np.float32)
    lgw = np.asarray(inputs["logits_w"], dtype=np.float32)
    lgb = np.asarray(inputs["logits_b"], dtype=np.float32)

    # fc_w^T packed: [N, FC] -> [128, KT*FC] with (p, nt, f)
    fwP = np.ascontiguousarray(
        fc_w.T.reshape(KT, P, FC).transpose(1, 0, 2).reshape(P, KT * FC)
    ).astype(BF)
    lgwT = np.ascontiguousarray(lgw.T).astype(BF)         # [FC, CLS]
    lgb_col = np.ascontiguousarray(lgb.reshape(CLS, 1))   # [CLS, 1]

    in_maps = []
    for i in range(NCORES):
        # x shard transposed + packed: [128, (mc, nt, j)]
        xT = x[i * BS : (i + 1) * BS].T                    # [N, BS]
        xP = np.ascontiguousarray(
            xT.reshape(KT, P, NMC, MCHUNK)
            .transpose(1, 2, 0, 3)
            .reshape(P, NMC * KT * MCHUNK)
        ).astype(BF)
        # W k-shard transposed + packed: [N, WROWS] -> [128, KT*WROWS]
        wT = W[i * WROWS : (i + 1) * WROWS].T              # [N, WROWS]
        wP = np.ascontiguousarray(
            wT.reshape(KT, P, WROWS).transpose(1, 0, 2).reshape(P, KT * WROWS)
        ).astype(BF)
        in_maps.append(
            {
                "xP": xP,
                "wP": wP,
                "fwP": fwP,
                "fc_b": fc_b,
                "lgwT": lgwT,
                "lgb": lgb_col,
            }
        )

    res = run_bass_kernel_spmd(
        nc,
        in_maps,
        core_ids=list(range(NCORES)),
        trace=bool(int(os.environ.get("KERNEL_TRACE", "0"))),
    )
    LAST_RESULT = res
    out = np.concatenate(
        [np.ascontiguousarray(r_["outT"].T) for r_ in res.results], axis=0
    )
    return out


# revision 20
# speedup vs baseline: 1.0164x; 1.0164x over previous
"""Trainium2 Bass kernel for nn_LDRFat (3-layer MLP forward).

reference: logits = relu((x @ W) @ fc_w.T + fc_b) @ logits_w.T + logits_b

Algebraic optimization: (x @ W) @ fc_w.T == x @ (W @ fc_w.T).
Precomputing Wfc = W @ fc_w.T ([3072,512]) collapses the dominant
309 GFLOP x@W matmul into a 51.5 GFLOP x@Wfc.

Layout strategy: ALL transposes/packing happen on the host (numpy)
inside kernel() — the device graph is a pure matmul stream (keeps the
PE HAM clock gate warm; transposes would idle it). Inputs are packed
into SBUF-layout panels [128, ...] so every DMA moves >=2.3KB
contiguous lines.

Phase A: each core computes its 384 rows of Wfc = W @ fc_w.T
(72 accumulating bf16 matmuls), then a bf16 AllGather replicates the
full Wfc. Phase B: h^T tiles [f, m] via matmuls contracted over n,
ReLU+bias on the scalar engine, then a 4-matmul logits head with
lgwT stationary producing out^T [10, m]; logits bias is applied as a
per-partition bias in the scalar-engine PSUM->SBUF copy. The final
output transpose ([10, BS] -> [BS, 10]) happens on the host.

Scheduling: the sync-engine DMA rings carry only phase-A inputs and x
chunks 0-1 up front, so the collective's input lands on idle rings
right after phase A. x chunks 2-3 and the gather-out unpacking ride
the activation engine's rings after the collective; the gather-in is
split per source core so phase B starts as soon as block 0 lands.
Dummy matmuls at t=0 pre-warm the PE clock (HAM gate).

All matmul operands are bf16 (fp32 PSUM accumulation): same PE rate as
fp32r but half the DMA/SBUF/collective bytes. Measured rel err ~5e-3
vs the 2e-2 gate.
"""

import os
import numpy as np
import ml_dtypes

import concourse.bass as bass
import concourse.mybir as mybir
import concourse.tile as tile
from concourse import bacc
from concourse.bass import MemorySpace, ts, ds
from concourse.bass_utils import run_bass_kernel_spmd

B = 16384
N = 3072
FC = 512
CLS = 10
NCORES = 8
BS = B // NCORES     # 2048 batch rows per core
P = 128

KT = N // P          # 24 contraction tiles
FT = FC // P         # 4 f-tiles
MCHUNK = 512
NMC = BS // MCHUNK   # 4 m-chunks per core
KSH = KT // NCORES   # 3 k-tiles per core in sharded precompute
WROWS = KSH * P      # 384 W-rows per core
NWARM = 24           # PE pre-warm matmuls

F32 = mybir.dt.float32
BF16 = mybir.dt.bfloat16
BF = ml_dtypes.bfloat16

_CACHE = {}
LAST_RESULT = None


def build_kernel():
    nc = bacc.Bacc(
        "TRN2",
        target_bir_lowering=False,
        debug=False,
        enable_asserts=False,
        num_devices=NCORES,
    )
    # All big inputs are host-packed to SBUF layout: [128 partitions, flat]
    xP_d = nc.dram_tensor("xP", [P, NMC * KT * MCHUNK], BF16, kind="ExternalInput").ap()
    wP_d = nc.dram_tensor("wP", [P, KT * WROWS], BF16, kind="ExternalInput").ap()
    fwP_d = nc.dram_tensor("fwP", [P, KT * FC], BF16, kind="ExternalInput").ap()
    fcb_d = nc.dram_tensor("fc_b", [FC], F32, kind="ExternalInput").ap()
    lwT_d = nc.dram_tensor("lgwT", [FC, CLS], BF16, kind="ExternalInput").ap()
    lgb_d = nc.dram_tensor("lgb", [CLS, 1], F32, kind="ExternalInput").ap()
    outT_d = nc.dram_tensor("outT", [CLS, BS], F32, kind="ExternalOutput").ap()

    with tile.TileContext(nc) as tc:
        with (
            tc.tile_pool(name="consts", bufs=1) as consts,
            tc.tile_pool(name="big", bufs=1) as big,
            tc.tile_pool(name="stage", bufs=1) as stage_pool,
            tc.tile_pool(name="yT_p", bufs=2) as yT_pool,
            tc.tile_pool(name="ps_a", bufs=2, space=MemorySpace.PSUM) as ps_a,
            tc.tile_pool(name="ps_b", bufs=4, space=MemorySpace.PSUM) as ps_b,
            tc.tile_pool(name="ps_lg", bufs=2, space=MemorySpace.PSUM) as ps_lg,
            tc.tile_pool(name="cc_dram", bufs=1, space=MemorySpace.DRAM) as ccd,
        ):
            fcb_sb = consts.tile([P, FT], F32)
            nc.sync.dma_start(fcb_sb, fcb_d.rearrange("(t p) -> p t", p=P))
            lgw_sb = consts.tile([P, FT, CLS], BF16)
            nc.sync.dma_start(lgw_sb, lwT_d.rearrange("(t p) c -> p t c", p=P))
            lgb_sb = consts.tile([CLS, 1], F32)
            nc.sync.dma_start(lgb_sb, lgb_d)

            # phase A inputs get the HBM to themselves (they gate the
            # collective, which gates phase B). All x loads are gated
            # behind gin below, so they stream during the collective's
            # dead window instead of competing with these.
            fwT_sb = big.tile([P, KT, FC], BF16)
            wT_sb = big.tile([P, KT, WROWS], BF16)
            for g in range(8):
                nc.sync.dma_start(
                    fwT_sb[:, 3 * g : 3 * g + 3],
                    fwP_d[:, ds(g * 3 * FC, 3 * FC)],
                )
                nc.sync.dma_start(
                    wT_sb[:, 3 * g : 3 * g + 3],
                    wP_d[:, ds(g * 3 * WROWS, 3 * WROWS)],
                )
            x_sb = [big.tile([P, KT, MCHUNK], BF16, name=f"x_{mc}") for mc in range(NMC)]
            XC = KT * MCHUNK  # flat elems per chunk

            def x_load(mc):
                for h in range(2):
                    nc.sync.dma_start(
                        x_sb[mc][:, 12 * h : 12 * h + 12],
                        xP_d[:, ds(mc * XC + h * XC // 2, XC // 2)],
                    )

            # x chunk 0 loads with the inputs: the collective trigger
            # requires total DMA quiescence (EQ-based semaphore waits),
            # so anything pre-collective must fully drain first — keep it
            # to the minimum that phase B needs immediately.
            x_load(0)

            # PE pre-warm: dummy matmuls (no data deps) fill the HAM
            # activity window so phase A runs at full clock.
            warm_f = consts.tile([P, MCHUNK], F32)
            nc.gpsimd.memset(warm_f, 0.01)
            warm_b = consts.tile([P, MCHUNK], BF16)
            nc.vector.tensor_copy(warm_b, warm_f)
            for i in range(NWARM):
                pw = ps_b.tile([P, MCHUNK], F32, tag="acc", name=f"warm_{i}")
                nc.tensor.matmul(pw, warm_b[:, :P], warm_b, start=True, stop=True)

            # ---------------- Phase A: this core's Wfc rows ----------------
            wfc_sb = big.tile([P, KT, FC], BF16)
            wfc_stage = stage_pool.tile([P, KSH, FC], BF16)
            for lkt in range(KSH):
                acc = ps_a.tile([P, FC], F32, tag="acc")
                for nt in range(KT):
                    nc.tensor.matmul(
                        acc,
                        wT_sb[:, nt, ts(lkt, P)],
                        fwT_sb[:, nt],
                        start=(nt == 0),
                        stop=(nt == KT - 1),
                    )
                nc.vector.tensor_copy(wfc_stage[:, lkt], acc)

            gin = ccd.tile([P, KSH * FC], BF16)
            nc.scalar.dma_start(gin, wfc_stage.rearrange("p a b -> p (a b)"))
            gout = ccd.tile([NCORES * P, KSH * FC], BF16)
            nc.gpsimd.collective_compute(
                "AllGather",
                mybir.AluOpType.bypass,
                replica_groups=[list(range(NCORES))],
                ins=[gin.opt()],
                outs=[gout.opt()],
            )
            # unpack per source core: block c holds global k-tiles
            # 3c..3c+2; phase B consumes kt in order, so it can start
            # as soon as block 0 lands.
            for c in range(NCORES):
                nc.scalar.dma_start(
                    wfc_sb[:, ds(KSH * c, KSH)].rearrange("p l f -> p (l f)"),
                    gout[ds(c * P, P), :],
                )
            # x chunk 1 streams once the collective has completed
            gate1 = ccd.tile([1, 2], BF16, name="gate1")
            nc.sync.dma_start(gate1, gout[0:1, 0:2])
            x_load(1)

            # ---------------- Phase B ----------------
            for mc in range(NMC):
                h2 = [
                    ps_b.tile([P, MCHUNK], F32, tag="acc", name=f"h2_{mc}_{ft}")
                    for ft in range(FT)
                ]
                for kt in range(KT):
                    for ft in range(FT):
                        nc.tensor.matmul(
                            h2[ft],
                            wfc_sb[:, kt, ts(ft, P)],
                            x_sb[mc][:, kt],
                            start=(kt == 0),
                            stop=(kt == KT - 1),
                        )

                # relu(h2 + fc_b) on the scalar engine, cast to bf16
                yT = yT_pool.tile([P, FT, MCHUNK], BF16, tag="yT")
                for ft in range(FT):
                    nc.scalar.activation(
                        yT[:, ft],
                        h2[ft],
                        mybir.ActivationFunctionType.Relu,
                        bias=fcb_sb[:, ds(ft, 1)],
                    )
                # release x chunk mc+2 while chunk mc+1 computes
                if mc + 2 < NMC:
                    gate = ccd.tile([1, 2], BF16, name=f"gate{mc + 2}")
                    nc.sync.dma_start(gate, yT[0:1, 0, 0:2])
                    x_load(mc + 2)

                # logits head: lgwT stationary, yT moving -> outT [CLS, m]
                plg = ps_lg.tile([CLS, MCHUNK], F32, tag="lg")
                for ft in range(FT):
                    nc.tensor.matmul(
                        plg,
                        lgw_sb[:, ft],
                        yT[:, ft],
                        start=(ft == 0),
                        stop=(ft == FT - 1),
                    )
                # + logits_b as per-partition bias during PSUM->SBUF copy
                osb = yT_pool.tile([CLS, MCHUNK], F32, tag="osb")
                nc.scalar.activation(
                    osb,
                    plg,
                    mybir.ActivationFunctionType.Identity,
                    bias=lgb_sb[:, ds(0, 1)],
                )
                nc.scalar.dma_start(outT_d[:, ts(mc, MCHUNK)], osb)

    nc.compile()
    return nc


def kernel(**inputs) -> np.ndarray:
    global LAST_RESULT
    if "nc" not in _CACHE:
        _CACHE["nc"] = build_kernel()
    nc = _CACHE["nc"]

    x = np.asarray(inputs["x"], dtype=np.float32)
    W = np.asarray(inputs["W"], dtype=np.float32)
    fc_w = np.asarray(inputs["fc_w"], dtype=np.float32)
    fc_b = np.asarray(inputs["fc_b"], dtype=np.float32)
    lgw = np.asarray(inputs["logits_w"], dtype=np.float32)
    lgb = np.asarray(inputs["logits_b"], dtype=np.float32)

    # fc_w^T packed: [N, FC] -> [128, KT*FC] with (p, nt, f)
    fwP = np.ascontiguousarray(
        fc_w.T.reshape(KT, P, FC).transpose(1, 0, 2).reshape(P, KT * FC)
    ).astype(BF)
    lgwT = np.ascontiguousarray(lgw.T).astype(BF)         # [FC, CLS]
    lgb_col = np.ascontiguousarray(lgb.reshape(CLS, 1))   # [CLS, 1]

    in_maps = []
    for i in range(NCORES):
        # x shard transposed + packed: [128, (mc, nt, j)]
        xT = x[i * BS : (i + 1) * BS].T                    # [N, BS]
        xP = np.ascontiguousarray(
            xT.reshape(KT, P, NMC, MCHUNK)
            .transpose(1, 2, 0, 3)
            .reshape(P, NMC * KT * MCHUNK)
        ).astype(BF)
        # W k-shard transposed + packed: [N, WROWS] -> [128, KT*WROWS]
        wT = W[i * WROWS : (i + 1) * WROWS].T              # [N, WROWS]
        wP = np.ascontiguousarray(
            wT.reshape(KT, P, WROWS).transpose(1, 0, 2).reshape(P, KT * WROWS)
        ).astype(BF)
        in_maps.append(
            {
                "xP": xP,
                "wP": wP,
                "fwP": fwP,
                "fc_b": fc_b,
                "lgwT": lgwT,
                "lgb": lgb_col,
            }
        )

    res = run_bass_kernel_spmd(
        nc,
        in_maps,
        core_ids=list(range(NCORES)),
        trace=bool(int(os.environ.get("KERNEL_TRACE", "0"))),
    )
    LAST_RESULT = res
    out = np.concatenate(
        [np.ascontiguousarray(r_["outT"].T) for r_ in res.results], axis=0
    )
    return out
